# revision 3
# baseline (speedup 1.0000x reference)
"""Trainium2 Bass kernel for nn_BasicBlockBase (sparse-conv BasicBlock, GNN message passing).

Computation (reference):
    x1 = sparse_conv(feats, idx, mask, W1) + b1       # b1 cancels in BN (training-mode)
    h  = relu(bn1(x1))
    x2 = sparse_conv(h, idx, mask, W2)
    out = relu(bn2(x2) + feats)

Strategy: data-parallel over points across 8 NeuronCores (12500 pts/core).
Per 128-point tile: one indirect-DMA gathers all 28*128 neighbor rows (bf16)
into an im2col tile G [128, 1792]; an SBUF->SBUF xbar DMA-transpose produces
G^T chunks [128, 14, 128]; 14 accumulating matmuls against reshaped weights
give the conv output transposed [64ch, 128pts] in PSUM.  BN statistics are
free-dim reductions, all-reduced across cores; BN is applied as a folded
per-channel affine relu(a*x+c).  conv2 requires the full activation table, so
shards are all-gathered into a replicated bf16 table.  Masked/padding
neighbors are pointed at a dedicated zero row of the table (no mask multiply).
"""
import sys

sys.path.insert(0, "/opt/trn_rl_repo")

import numpy as np
import ml_dtypes
from contextlib import ExitStack

import concourse.bass as bass
import concourse.tile as tile
import concourse.bacc as bacc
from concourse import mybir

BF16 = mybir.dt.bfloat16
F32 = mybir.dt.float32
I32 = mybir.dt.int32
bf16 = ml_dtypes.bfloat16

P = 128          # points per tile / partition count
C = 64           # channels
K = 27           # kernel offsets
KS = 28          # k slots (27 real + 1 zero pad)
NCHUNK = KS * C // P   # 14 contraction chunks of 128


class Cfg:
    def __init__(self, n_points, n_cores):
        self.N = n_points
        self.M = n_cores
        assert n_points % n_cores == 0
        self.SHARD = n_points // n_cores
        self.TILES = (self.SHARD + P - 1) // P
        self.SHARD_PAD = self.TILES * P
        # feats table: N rows + zero rows padded to multiple of 32
        self.Z1 = self.N                       # zero row index in feats table
        self.FT_ROWS = self.N + 32
        # h table: all-gathered shards (SHARD_PAD each) + zero rows
        self.Z2 = self.M * self.SHARD_PAD      # zero row index in h table
        self.HT_ROWS = self.Z2 + 32


CFG_FULL = Cfg(100000, 8)


def build_kernel(cfg: Cfg):
    """Build the Bass module for one SPMD core. Returns compiled nc."""
    M, TILES, SHARD_PAD = cfg.M, cfg.TILES, cfg.SHARD_PAD

    nc = bacc.Bacc("TRN2", target_bir_lowering=False, debug=False, num_devices=M)

    ft = nc.dram_tensor("ft", [cfg.FT_ROWS, C], BF16, kind="ExternalInput")
    fres = nc.dram_tensor("fres", [SHARD_PAD, C], F32, kind="ExternalInput")
    i1 = nc.dram_tensor("i1", [SHARD_PAD, KS], I32, kind="ExternalInput")
    i2 = nc.dram_tensor("i2", [SHARD_PAD, KS], I32, kind="ExternalInput")
    w1 = nc.dram_tensor("w1", [P, NCHUNK * C], BF16, kind="ExternalInput")
    w2 = nc.dram_tensor("w2", [P, NCHUNK * C], BF16, kind="ExternalInput")
    gb1 = nc.dram_tensor("gb1", [C, 2], F32, kind="ExternalInput")
    gb2 = nc.dram_tensor("gb2", [C, 2], F32, kind="ExternalInput")
    out_d = nc.dram_tensor("out", [SHARD_PAD, C], F32, kind="ExternalOutput")

    h_shard = nc.dram_tensor("h_shard", [SHARD_PAD, C], BF16)
    h_tbl = nc.dram_tensor("h_tbl", [cfg.HT_ROWS, C], BF16, addr_space="Shared")
    st1_loc = nc.dram_tensor("st1_loc", [C, 2], F32)
    st1_red = nc.dram_tensor("st1_red", [C, 2], F32, addr_space="Shared")
    st2_loc = nc.dram_tensor("st2_loc", [C, 2], F32)
    st2_red = nc.dram_tensor("st2_red", [C, 2], F32, addr_space="Shared")

    groups = [list(range(M))]
    inv_n = 1.0 / float(cfg.N)

    with tile.TileContext(nc) as tc, ExitStack() as ctx:
        persist = ctx.enter_context(tc.tile_pool(name="persist", bufs=1))
        idxp = ctx.enter_context(tc.tile_pool(name="idxp", bufs=4))
        gp = ctx.enter_context(tc.tile_pool(name="gp", bufs=3))
        gtp = ctx.enter_context(tc.tile_pool(name="gtp", bufs=3))
        psp = ctx.enter_context(tc.tile_pool(name="psp", bufs=4, space="PSUM"))
        smallp = ctx.enter_context(tc.tile_pool(name="smallp", bufs=4))
        rowsp = ctx.enter_context(tc.tile_pool(name="rowsp", bufs=4))

        w1_sb = persist.tile([P, NCHUNK * C], BF16)
        nc.sync.dma_start(out=w1_sb[:], in_=w1[:, :])
        w2_sb = persist.tile([P, NCHUNK * C], BF16)
        nc.sync.dma_start(out=w2_sb[:], in_=w2[:, :])
        gb1_sb = persist.tile([C, 2], F32)
        nc.sync.dma_start(out=gb1_sb[:], in_=gb1[:, :])
        gb2_sb = persist.tile([C, 2], F32)
        nc.sync.dma_start(out=gb2_sb[:], in_=gb2[:, :])

        xt_full = persist.tile([C, SHARD_PAD], BF16)
        yt_full = persist.tile([C, SHARD_PAD], BF16)
        acc1 = persist.tile([C, TILES], F32)
        acc2 = persist.tile([C, TILES], F32)
        acc3 = persist.tile([C, TILES], F32)
        acc4 = persist.tile([C, TILES], F32)

        # zero rows of h table (written once; collective writes rows [0, Z2))
        zrow = persist.tile([32, C], BF16)
        nc.vector.memset(zrow[:], 0)
        nc.sync.dma_start(out=h_tbl[cfg.Z2:cfg.Z2 + 32, :], in_=zrow[:])

        def conv_phase(idx_d, tbl_d, w_sb, acc_s, acc_q, xt_dst):
            for t in range(TILES):
                ts = slice(t * P, (t + 1) * P)
                idx_t = idxp.tile([P, KS], I32)
                nc.sync.dma_start(out=idx_t[:], in_=idx_d[ts, :])
                g = gp.tile([P, KS * C], BF16)
                # HW indirect DMA uses one index per partition: one gather per
                # k-slot, each fetching 128 rows. Slot K (pad) is zero-filled.
                nc.vector.memset(g[:, K * C:KS * C], 0)
                for k in range(K):
                    nc.gpsimd.indirect_dma_start(
                        out=g[:, k * C:(k + 1) * C], out_offset=None,
                        in_=tbl_d[:, :],
                        in_offset=bass.IndirectOffsetOnAxis(
                            ap=idx_t[:, k:k + 1], axis=0),
                    )
                gt = gtp.tile([P, NCHUNK, P], BF16)
                nc.sync.dma_start(out=gt[:], in_=g[:], transpose=True)
                xt_ps = psp.tile([C, P], F32, space="PSUM")
                for j in range(NCHUNK):
                    nc.tensor.matmul(
                        out=xt_ps[:],
                        lhsT=w_sb[:, j * C:(j + 1) * C],
                        rhs=gt[:, j, :],
                        start=(j == 0),
                        stop=(j == NCHUNK - 1),
                    )
                # copy+cast to bf16 column block, fused sum -> acc_s[:, t]
                nc.scalar.activation(
                    out=xt_dst[:, ts], in_=xt_ps[:],
                    func=mybir.ActivationFunctionType.Copy,
                    accum_out=acc_s[:, t:t + 1],
                )
                # x^2 with fused sum -> acc_q[:, t] (reads the SBUF bf16 copy;
                # only one PSUM operand is allowed per DVE instruction)
                x2 = smallp.tile([C, P], BF16, tag="x2")
                nc.vector.scalar_tensor_tensor(
                    out=x2[:], in0=xt_dst[:, ts], scalar=1.0, in1=xt_dst[:, ts],
                    op0=mybir.AluOpType.mult, op1=mybir.AluOpType.mult,
                    accum_out=acc_q[:, t:t + 1],
                )

        def bn_coeffs(acc_s, acc_q, st_loc, st_red, gb_sb):
            """All-reduce per-channel sums, return (a, c) affine tiles [C,1]."""
            stats = smallp.tile([C, 2], F32, tag="stats")
            nc.vector.tensor_reduce(
                out=stats[:, 0:1], in_=acc_s[:], axis=mybir.AxisListType.X,
                op=mybir.AluOpType.add)
            nc.vector.tensor_reduce(
                out=stats[:, 1:2], in_=acc_q[:], axis=mybir.AxisListType.X,
                op=mybir.AluOpType.add)
            nc.sync.dma_start(out=st_loc[:, :], in_=stats[:])
            nc.gpsimd.collective_compute(
                "AllReduce", mybir.AluOpType.add, replica_groups=groups,
                ins=[st_loc.ap().opt()], outs=[st_red.ap().opt()],
            )
            red = smallp.tile([C, 2], F32, tag="red")
            nc.sync.dma_start(out=red[:], in_=st_red[:, :])
            mean = smallp.tile([C, 1], F32, tag="mean")
            nc.vector.tensor_scalar_mul(mean[:], red[:, 0:1], inv_n)
            var = smallp.tile([C, 1], F32, tag="var")
            # var = E[x^2] - mean^2 + eps = (red1*inv_n - mean*mean) + eps
            nc.vector.scalar_tensor_tensor(
                out=var[:], in0=mean[:], scalar=-1.0, in1=mean[:],
                op0=mybir.AluOpType.mult, op1=mybir.AluOpType.mult)   # -mean^2
            nc.vector.scalar_tensor_tensor(
                out=var[:], in0=red[:, 1:2], scalar=inv_n, in1=var[:],
                op0=mybir.AluOpType.mult, op1=mybir.AluOpType.add)
            nc.vector.tensor_scalar_add(var[:], var[:], 1e-5)
            sq = smallp.tile([C, 1], F32, tag="sq")
            nc.scalar.activation(out=sq[:], in_=var[:],
                                 func=mybir.ActivationFunctionType.Sqrt)
            rsq = smallp.tile([C, 1], F32, tag="rsq")
            nc.vector.reciprocal(out=rsq[:], in_=sq[:])
            a = persist.tile([C, 1], F32, tag=f"a_{st_loc.name}")
            nc.vector.tensor_mul(out=a[:], in0=gb_sb[:, 0:1], in1=rsq[:])
            c = persist.tile([C, 1], F32, tag=f"c_{st_loc.name}")
            # c = beta - mean*a
            nc.vector.scalar_tensor_tensor(
                out=c[:], in0=mean[:], scalar=-1.0, in1=a[:],
                op0=mybir.AluOpType.mult, op1=mybir.AluOpType.mult)  # -mean*a
            nc.vector.tensor_add(out=c[:], in0=gb_sb[:, 1:2], in1=c[:])
            return a, c

        # ---- conv1 ----
        conv_phase(i1, ft, w1_sb, acc1, acc2, xt_full)
        a1, c1 = bn_coeffs(acc1, acc2, st1_loc, st1_red, gb1_sb)

        # ---- h = relu(a1*x + c1), transpose to rows, write shard ----
        for t in range(TILES):
            ts = slice(t * P, (t + 1) * P)
            hbuf = smallp.tile([C, P], BF16, tag="hbuf")
            nc.scalar.activation(
                out=hbuf[:], in_=xt_full[:, ts],
                func=mybir.ActivationFunctionType.Relu,
                bias=c1[:], scale=a1[:])
            hrows = rowsp.tile([P, C], BF16, tag="hrows")
            nc.sync.dma_start(out=hrows[:], in_=hbuf[:], transpose=True)
            nc.sync.dma_start(out=h_shard[ts, :], in_=hrows[:])

        nc.gpsimd.collective_compute(
            "AllGather", mybir.AluOpType.bypass, replica_groups=groups,
            ins=[h_shard.ap().opt()],
            outs=[h_tbl[0:cfg.Z2, :].opt()],
        )

        # ---- conv2 ----
        conv_phase(i2, h_tbl, w2_sb, acc3, acc4, yt_full)
        a2, c2 = bn_coeffs(acc3, acc4, st2_loc, st2_red, gb2_sb)

        # ---- out = relu(a2*y + c2 + feats) ----
        for t in range(TILES):
            ts = slice(t * P, (t + 1) * P)
            zbuf = smallp.tile([C, P], BF16, tag="zbuf")
            nc.vector.tensor_scalar(
                out=zbuf[:], in0=yt_full[:, ts], scalar1=a2[:], scalar2=c2[:],
                op0=mybir.AluOpType.mult, op1=mybir.AluOpType.add)
            zrows = rowsp.tile([P, C], BF16, tag="zrows")
            nc.sync.dma_start(out=zrows[:], in_=zbuf[:], transpose=True)
            f_t = rowsp.tile([P, C], F32, tag="ft_res")
            nc.sync.dma_start(out=f_t[:], in_=fres[ts, :])
            o_t = rowsp.tile([P, C], F32, tag="ot")
            nc.vector.tensor_add(out=o_t[:], in0=zrows[:], in1=f_t[:])
            nc.vector.tensor_scalar_max(o_t[:], o_t[:], 0.0)
            nc.sync.dma_start(out=out_d[ts, :], in_=o_t[:])

    nc.compile()
    return nc


def prep_inputs(cfg: Cfg, feats, W1, b1, gamma1, beta1, W2, gamma2, beta2,
                nbr_idx, nbr_mask):
    """Host-side sharding/layout prep. Returns list of per-core input dicts."""
    N, M, SHARD, SHARD_PAD = cfg.N, cfg.M, cfg.SHARD, cfg.SHARD_PAD

    ft = np.zeros((cfg.FT_ROWS, C), dtype=bf16)
    ft[:N] = feats.astype(bf16)

    def wprep(W):
        Wf = np.zeros((NCHUNK * P, C), dtype=np.float32)
        Wf[:K * C] = W.reshape(K * C, C)
        return np.ascontiguousarray(
            Wf.reshape(NCHUNK, P, C).transpose(1, 0, 2).reshape(P, NCHUNK * C)
        ).astype(bf16)

    w1p, w2p = wprep(W1), wprep(W2)
    gb1 = np.stack([np.asarray(gamma1, np.float32),
                    np.asarray(beta1, np.float32)], axis=1)
    gb2 = np.stack([np.asarray(gamma2, np.float32),
                    np.asarray(beta2, np.float32)], axis=1)

    mask = np.asarray(nbr_mask, bool)
    idx = np.asarray(nbr_idx, np.int64)
    i1_full = np.where(mask, idx, cfg.Z1).astype(np.int32)
    i2_full = np.where(mask, (idx // SHARD) * SHARD_PAD + idx % SHARD,
                       cfg.Z2).astype(np.int32)

    in_maps = []
    for c in range(M):
        rows = slice(c * SHARD, (c + 1) * SHARD)
        i1c = np.full((SHARD_PAD, KS), cfg.Z1, np.int32)
        i1c[:SHARD, :K] = i1_full[rows]
        i2c = np.full((SHARD_PAD, KS), cfg.Z2, np.int32)
        i2c[:SHARD, :K] = i2_full[rows]
        fres = np.zeros((SHARD_PAD, C), np.float32)
        fres[:SHARD] = feats[rows]
        in_maps.append({
            "ft": ft, "fres": fres, "i1": i1c, "i2": i2c,
            "w1": w1p, "w2": w2p, "gb1": gb1, "gb2": gb2,
        })
    return in_maps


_NC_CACHE = {}


def _get_nc(cfg: Cfg):
    key = (cfg.N, cfg.M)
    if key not in _NC_CACHE:
        _NC_CACHE[key] = build_kernel(cfg)
    return _NC_CACHE[key]


def kernel(feats, W1, b1, gamma1, beta1, W2, gamma2, beta2, nbr_idx, nbr_mask):
    from concourse.bass_utils import run_bass_kernel_spmd

    cfg = CFG_FULL
    nc = _get_nc(cfg)
    in_maps = prep_inputs(cfg, np.asarray(feats, np.float32),
                          np.asarray(W1, np.float32), np.asarray(b1, np.float32),
                          np.asarray(gamma1, np.float32), np.asarray(beta1, np.float32),
                          np.asarray(W2, np.float32),
                          np.asarray(gamma2, np.float32), np.asarray(beta2, np.float32),
                          np.asarray(nbr_idx), np.asarray(nbr_mask))
    res = run_bass_kernel_spmd(nc, in_maps, core_ids=list(range(cfg.M)))
    out = np.concatenate([res.results[c]["out"][:cfg.SHARD] for c in range(cfg.M)],
                         axis=0)
    return out.astype(np.float32)


# revision 5
# speedup vs baseline: 1.1447x; 1.1447x over previous
"""Trainium2 Bass kernel for nn_BasicBlockBase (sparse-conv BasicBlock, GNN message passing).

Computation (reference):
    x1 = sparse_conv(feats, idx, mask, W1) + b1       # b1 cancels in BN (training-mode)
    h  = relu(bn1(x1))
    x2 = sparse_conv(h, idx, mask, W2)
    out = relu(bn2(x2) + feats)

Strategy: data-parallel over points across 8 NeuronCores (12500 pts/core).
Per 128-point tile: one indirect-DMA gathers all 28*128 neighbor rows (bf16)
into an im2col tile G [128, 1792]; an SBUF->SBUF xbar DMA-transpose produces
G^T chunks [128, 14, 128]; 14 accumulating matmuls against reshaped weights
give the conv output transposed [64ch, 128pts] in PSUM.  BN statistics are
free-dim reductions, all-reduced across cores; BN is applied as a folded
per-channel affine relu(a*x+c).  conv2 requires the full activation table, so
shards are all-gathered into a replicated bf16 table.  Masked/padding
neighbors are pointed at a dedicated zero row of the table (no mask multiply).
"""
import sys

sys.path.insert(0, "/opt/trn_rl_repo")

import numpy as np
import ml_dtypes
from contextlib import ExitStack

import concourse.bass as bass
import concourse.tile as tile
import concourse.bacc as bacc
from concourse import mybir

BF16 = mybir.dt.bfloat16
F32 = mybir.dt.float32
I32 = mybir.dt.int32
bf16 = ml_dtypes.bfloat16

P = 128          # points per tile / partition count
C = 64           # channels
K = 27           # kernel offsets
KS = 28          # k slots (27 real + 1 zero pad)
NCHUNK = KS * C // P   # 14 contraction chunks of 128


class Cfg:
    def __init__(self, n_points, n_cores):
        self.N = n_points
        self.M = n_cores
        assert n_points % n_cores == 0
        self.SHARD = n_points // n_cores
        self.TILES = (self.SHARD + P - 1) // P
        self.SHARD_PAD = self.TILES * P
        # feats table: N rows + zero rows padded to multiple of 32
        self.Z1 = self.N                       # zero row index in feats table
        self.FT_ROWS = self.N + 32
        # h table: all-gathered shards (SHARD_PAD each) + zero rows
        self.Z2 = self.M * self.SHARD_PAD      # zero row index in h table
        self.HT_ROWS = self.Z2 + 32


CFG_FULL = Cfg(100000, 8)


def build_kernel(cfg: Cfg):
    """Build the Bass module for one SPMD core. Returns compiled nc."""
    M, TILES, SHARD_PAD = cfg.M, cfg.TILES, cfg.SHARD_PAD

    nc = bacc.Bacc("TRN2", target_bir_lowering=False, debug=False, num_devices=M)

    ft = nc.dram_tensor("ft", [cfg.FT_ROWS, C], BF16, kind="ExternalInput")
    fres = nc.dram_tensor("fres", [SHARD_PAD, C], F32, kind="ExternalInput")
    i1 = nc.dram_tensor("i1", [SHARD_PAD, KS], I32, kind="ExternalInput")
    i2 = nc.dram_tensor("i2", [SHARD_PAD, KS], I32, kind="ExternalInput")
    w1 = nc.dram_tensor("w1", [P, NCHUNK * C], BF16, kind="ExternalInput")
    w2 = nc.dram_tensor("w2", [P, NCHUNK * C], BF16, kind="ExternalInput")
    gb1 = nc.dram_tensor("gb1", [C, 2], F32, kind="ExternalInput")
    gb2 = nc.dram_tensor("gb2", [C, 2], F32, kind="ExternalInput")
    out_d = nc.dram_tensor("out", [SHARD_PAD, C], F32, kind="ExternalOutput")

    h_shard = nc.dram_tensor("h_shard", [SHARD_PAD, C], BF16)
    h_tbl = nc.dram_tensor("h_tbl", [cfg.HT_ROWS, C], BF16, addr_space="Shared")
    st1_loc = nc.dram_tensor("st1_loc", [C, 2], F32)
    st1_red = nc.dram_tensor("st1_red", [C, 2], F32, addr_space="Shared")
    st2_loc = nc.dram_tensor("st2_loc", [C, 2], F32)
    st2_red = nc.dram_tensor("st2_red", [C, 2], F32, addr_space="Shared")

    groups = [list(range(M))]
    inv_n = 1.0 / float(cfg.N)

    with tile.TileContext(nc) as tc, ExitStack() as ctx:
        persist = ctx.enter_context(tc.tile_pool(name="persist", bufs=1))
        idxp = ctx.enter_context(tc.tile_pool(name="idxp", bufs=4))
        gp = ctx.enter_context(tc.tile_pool(name="gp", bufs=3))
        gtp = ctx.enter_context(tc.tile_pool(name="gtp", bufs=3))
        psp = ctx.enter_context(tc.tile_pool(name="psp", bufs=4, space="PSUM"))
        tpsp = ctx.enter_context(tc.tile_pool(name="tpsp", bufs=4, space="PSUM"))
        smallp = ctx.enter_context(tc.tile_pool(name="smallp", bufs=4))
        rowsp = ctx.enter_context(tc.tile_pool(name="rowsp", bufs=4))

        from concourse.masks import make_identity
        ident = persist.tile([P, P], BF16)
        make_identity(nc, ident[:])

        w1_sb = persist.tile([P, NCHUNK * C], BF16)
        nc.sync.dma_start(out=w1_sb[:], in_=w1[:, :])
        w2_sb = persist.tile([P, NCHUNK * C], BF16)
        nc.sync.dma_start(out=w2_sb[:], in_=w2[:, :])
        gb1_sb = persist.tile([C, 2], F32)
        nc.sync.dma_start(out=gb1_sb[:], in_=gb1[:, :])
        gb2_sb = persist.tile([C, 2], F32)
        nc.sync.dma_start(out=gb2_sb[:], in_=gb2[:, :])

        xt_full = persist.tile([C, SHARD_PAD], BF16)
        yt_full = persist.tile([C, SHARD_PAD], BF16)
        acc1 = persist.tile([C, TILES], F32)
        acc2 = persist.tile([C, TILES], F32)
        acc3 = persist.tile([C, TILES], F32)
        acc4 = persist.tile([C, TILES], F32)

        # zero rows of h table (written once; collective writes rows [0, Z2))
        zrow = persist.tile([32, C], BF16)
        nc.vector.memset(zrow[:], 0)
        nc.sync.dma_start(out=h_tbl[cfg.Z2:cfg.Z2 + 32, :], in_=zrow[:])

        def conv_phase(idx_d, tbl_d, w_sb, acc_s, acc_q, xt_dst):
            for t in range(TILES):
                ts = slice(t * P, (t + 1) * P)
                idx_t = idxp.tile([P, KS], I32)
                nc.sync.dma_start(out=idx_t[:], in_=idx_d[ts, :])
                g = gp.tile([P, KS * C], BF16)
                # HW indirect DMA uses one index per partition: one gather per
                # k-slot, each fetching 128 rows. Slot K (pad) is zero-filled.
                nc.vector.memset(g[:, K * C:KS * C], 0)
                for k in range(K):
                    nc.gpsimd.indirect_dma_start(
                        out=g[:, k * C:(k + 1) * C], out_offset=None,
                        in_=tbl_d[:, :],
                        in_offset=bass.IndirectOffsetOnAxis(
                            ap=idx_t[:, k:k + 1], axis=0),
                    )
                # transpose G chunk-by-chunk on PE (the SBUF->SBUF xbar DMA
                # runs on a single SDMA engine at ~34GB/s and stalls the
                # gather pipeline); copies PSUM->SBUF split across ACT/DVE.
                gt = gtp.tile([P, NCHUNK, P], BF16)
                xt_ps = psp.tile([C, P], F32, space="PSUM")
                for j in range(NCHUNK):
                    gt_ps = tpsp.tile([P, P], BF16, space="PSUM", tag="gt_ps")
                    nc.tensor.transpose(
                        out=gt_ps[:], in_=g[:, j * P:(j + 1) * P],
                        identity=ident[:])
                    if j % 2 == 0:
                        nc.scalar.activation(
                            out=gt[:, j, :], in_=gt_ps[:],
                            func=mybir.ActivationFunctionType.Copy)
                    else:
                        nc.vector.tensor_copy(out=gt[:, j, :], in_=gt_ps[:])
                    nc.tensor.matmul(
                        out=xt_ps[:],
                        lhsT=w_sb[:, j * C:(j + 1) * C],
                        rhs=gt[:, j, :],
                        start=(j == 0),
                        stop=(j == NCHUNK - 1),
                    )
                # copy+cast to bf16 column block, fused sum -> acc_s[:, t]
                nc.scalar.activation(
                    out=xt_dst[:, ts], in_=xt_ps[:],
                    func=mybir.ActivationFunctionType.Copy,
                    accum_out=acc_s[:, t:t + 1],
                )
                # x^2 with fused sum -> acc_q[:, t] (reads the SBUF bf16 copy;
                # only one PSUM operand is allowed per DVE instruction)
                x2 = smallp.tile([C, P], BF16, tag="x2")
                nc.vector.scalar_tensor_tensor(
                    out=x2[:], in0=xt_dst[:, ts], scalar=1.0, in1=xt_dst[:, ts],
                    op0=mybir.AluOpType.mult, op1=mybir.AluOpType.mult,
                    accum_out=acc_q[:, t:t + 1],
                )

        def bn_coeffs(acc_s, acc_q, st_loc, st_red, gb_sb):
            """All-reduce per-channel sums, return (a, c) affine tiles [C,1]."""
            stats = smallp.tile([C, 2], F32, tag="stats")
            nc.vector.tensor_reduce(
                out=stats[:, 0:1], in_=acc_s[:], axis=mybir.AxisListType.X,
                op=mybir.AluOpType.add)
            nc.vector.tensor_reduce(
                out=stats[:, 1:2], in_=acc_q[:], axis=mybir.AxisListType.X,
                op=mybir.AluOpType.add)
            nc.sync.dma_start(out=st_loc[:, :], in_=stats[:])
            nc.gpsimd.collective_compute(
                "AllReduce", mybir.AluOpType.add, replica_groups=groups,
                ins=[st_loc.ap().opt()], outs=[st_red.ap().opt()],
            )
            red = smallp.tile([C, 2], F32, tag="red")
            nc.sync.dma_start(out=red[:], in_=st_red[:, :])
            mean = smallp.tile([C, 1], F32, tag="mean")
            nc.vector.tensor_scalar_mul(mean[:], red[:, 0:1], inv_n)
            var = smallp.tile([C, 1], F32, tag="var")
            # var = E[x^2] - mean^2 + eps = (red1*inv_n - mean*mean) + eps
            nc.vector.scalar_tensor_tensor(
                out=var[:], in0=mean[:], scalar=-1.0, in1=mean[:],
                op0=mybir.AluOpType.mult, op1=mybir.AluOpType.mult)   # -mean^2
            nc.vector.scalar_tensor_tensor(
                out=var[:], in0=red[:, 1:2], scalar=inv_n, in1=var[:],
                op0=mybir.AluOpType.mult, op1=mybir.AluOpType.add)
            nc.vector.tensor_scalar_add(var[:], var[:], 1e-5)
            sq = smallp.tile([C, 1], F32, tag="sq")
            nc.scalar.activation(out=sq[:], in_=var[:],
                                 func=mybir.ActivationFunctionType.Sqrt)
            rsq = smallp.tile([C, 1], F32, tag="rsq")
            nc.vector.reciprocal(out=rsq[:], in_=sq[:])
            a = persist.tile([C, 1], F32, tag=f"a_{st_loc.name}")
            nc.vector.tensor_mul(out=a[:], in0=gb_sb[:, 0:1], in1=rsq[:])
            c = persist.tile([C, 1], F32, tag=f"c_{st_loc.name}")
            # c = beta - mean*a
            nc.vector.scalar_tensor_tensor(
                out=c[:], in0=mean[:], scalar=-1.0, in1=a[:],
                op0=mybir.AluOpType.mult, op1=mybir.AluOpType.mult)  # -mean*a
            nc.vector.tensor_add(out=c[:], in0=gb_sb[:, 1:2], in1=c[:])
            return a, c

        # ---- conv1 ----
        conv_phase(i1, ft, w1_sb, acc1, acc2, xt_full)
        a1, c1 = bn_coeffs(acc1, acc2, st1_loc, st1_red, gb1_sb)

        # ---- h = relu(a1*x + c1), transpose to rows, write shard ----
        for t in range(TILES):
            ts = slice(t * P, (t + 1) * P)
            hbuf = smallp.tile([C, P], BF16, tag="hbuf")
            nc.scalar.activation(
                out=hbuf[:], in_=xt_full[:, ts],
                func=mybir.ActivationFunctionType.Relu,
                bias=c1[:], scale=a1[:])
            hrows = rowsp.tile([P, C], BF16, tag="hrows")
            nc.sync.dma_start(out=hrows[:], in_=hbuf[:], transpose=True)
            nc.sync.dma_start(out=h_shard[ts, :], in_=hrows[:])

        nc.gpsimd.collective_compute(
            "AllGather", mybir.AluOpType.bypass, replica_groups=groups,
            ins=[h_shard.ap().opt()],
            outs=[h_tbl[0:cfg.Z2, :].opt()],
        )

        # ---- conv2 ----
        conv_phase(i2, h_tbl, w2_sb, acc3, acc4, yt_full)
        a2, c2 = bn_coeffs(acc3, acc4, st2_loc, st2_red, gb2_sb)

        # ---- out = relu(a2*y + c2 + feats) ----
        for t in range(TILES):
            ts = slice(t * P, (t + 1) * P)
            zbuf = smallp.tile([C, P], BF16, tag="zbuf")
            nc.vector.tensor_scalar(
                out=zbuf[:], in0=yt_full[:, ts], scalar1=a2[:], scalar2=c2[:],
                op0=mybir.AluOpType.mult, op1=mybir.AluOpType.add)
            zrows = rowsp.tile([P, C], BF16, tag="zrows")
            nc.sync.dma_start(out=zrows[:], in_=zbuf[:], transpose=True)
            f_t = rowsp.tile([P, C], F32, tag="ft_res")
            nc.sync.dma_start(out=f_t[:], in_=fres[ts, :])
            o_t = rowsp.tile([P, C], F32, tag="ot")
            nc.vector.tensor_add(out=o_t[:], in0=zrows[:], in1=f_t[:])
            nc.vector.tensor_scalar_max(o_t[:], o_t[:], 0.0)
            nc.sync.dma_start(out=out_d[ts, :], in_=o_t[:])

    nc.compile()
    return nc


def prep_inputs(cfg: Cfg, feats, W1, b1, gamma1, beta1, W2, gamma2, beta2,
                nbr_idx, nbr_mask):
    """Host-side sharding/layout prep. Returns list of per-core input dicts."""
    N, M, SHARD, SHARD_PAD = cfg.N, cfg.M, cfg.SHARD, cfg.SHARD_PAD

    ft = np.zeros((cfg.FT_ROWS, C), dtype=bf16)
    ft[:N] = feats.astype(bf16)

    def wprep(W):
        Wf = np.zeros((NCHUNK * P, C), dtype=np.float32)
        Wf[:K * C] = W.reshape(K * C, C)
        return np.ascontiguousarray(
            Wf.reshape(NCHUNK, P, C).transpose(1, 0, 2).reshape(P, NCHUNK * C)
        ).astype(bf16)

    w1p, w2p = wprep(W1), wprep(W2)
    gb1 = np.stack([np.asarray(gamma1, np.float32),
                    np.asarray(beta1, np.float32)], axis=1)
    gb2 = np.stack([np.asarray(gamma2, np.float32),
                    np.asarray(beta2, np.float32)], axis=1)

    mask = np.asarray(nbr_mask, bool)
    idx = np.asarray(nbr_idx, np.int64)
    i1_full = np.where(mask, idx, cfg.Z1).astype(np.int32)
    i2_full = np.where(mask, (idx // SHARD) * SHARD_PAD + idx % SHARD,
                       cfg.Z2).astype(np.int32)

    in_maps = []
    for c in range(M):
        rows = slice(c * SHARD, (c + 1) * SHARD)
        i1c = np.full((SHARD_PAD, KS), cfg.Z1, np.int32)
        i1c[:SHARD, :K] = i1_full[rows]
        i2c = np.full((SHARD_PAD, KS), cfg.Z2, np.int32)
        i2c[:SHARD, :K] = i2_full[rows]
        fres = np.zeros((SHARD_PAD, C), np.float32)
        fres[:SHARD] = feats[rows]
        in_maps.append({
            "ft": ft, "fres": fres, "i1": i1c, "i2": i2c,
            "w1": w1p, "w2": w2p, "gb1": gb1, "gb2": gb2,
        })
    return in_maps


_NC_CACHE = {}


def _get_nc(cfg: Cfg):
    key = (cfg.N, cfg.M)
    if key not in _NC_CACHE:
        _NC_CACHE[key] = build_kernel(cfg)
    return _NC_CACHE[key]


def kernel(feats, W1, b1, gamma1, beta1, W2, gamma2, beta2, nbr_idx, nbr_mask):
    from concourse.bass_utils import run_bass_kernel_spmd

    cfg = CFG_FULL
    nc = _get_nc(cfg)
    in_maps = prep_inputs(cfg, np.asarray(feats, np.float32),
                          np.asarray(W1, np.float32), np.asarray(b1, np.float32),
                          np.asarray(gamma1, np.float32), np.asarray(beta1, np.float32),
                          np.asarray(W2, np.float32),
                          np.asarray(gamma2, np.float32), np.asarray(beta2, np.float32),
                          np.asarray(nbr_idx), np.asarray(nbr_mask))
    res = run_bass_kernel_spmd(nc, in_maps, core_ids=list(range(cfg.M)))
    out = np.concatenate([res.results[c]["out"][:cfg.SHARD] for c in range(cfg.M)],
                         axis=0)
    return out.astype(np.float32)
